# revision 11
# baseline (speedup 1.0000x reference)
"""CVRP decoder kernel for 8 Trainium2 NeuronCores (pure data parallel).

Computes, per batch b:
    k = enc @ Wk.T ; v = enc @ Wv.T ; q = [eln, load] @ Wq.T
    eb = exp(-a1*ls*cur_dist + mask)
    weighted = (eb @ (exp(k)*v)) / (eb @ exp(k))
    aafm = sigmoid(q) * weighted
    score = aafm @ enc.T
    probs = softmax(10*tanh(score/sqrt(D) - a2*ls*cur_dist) + mask)

Sharding: batch (128) split across 8 cores, 16 batches/core. Weights are
replicated. Each core runs an identical Bass program (SPMD, no collectives).

v3 layout (vs the transpose-heavy v1): cur_dist is uploaded TWICE in bf16 —
once host-transposed (m on partitions, for the exp-bias path) and once
natural (n on partitions, for the score bias) — same HBM bytes as one f32
copy, and the 16 per-batch 128x128 PE transposes disappear entirely.  All
matmul operands are bf16 (f32 PSUM accumulation); end-to-end quantization
error is ~5e-3 against the f64 reference, well inside the 2e-2 gate.

  - n-axis uses the slot permutation n = 4p + c everywhere (partition-major
    DMA: 4 contiguous rows per partition); m-axis is natural (cdT DMA uses
    1KB-per-row descriptors, chunk c <-> m = 128c + p).
  - eb^T = one 2048-wide ACT exp straight from the DMA'd cdT tile.
  - score bias -sqrt(D)*c2*cd folds into score PSUM via a scaled-identity
    matmul against the natural cd tile (or DVE stt, cfg "fold").
  - softmax stays in natural (n, m) layout: free-dim reduce on DVE, final
    scale on the otherwise-idle GpSimd engine.
  - sigmoid(x) = 0.5 + 0.5*tanh(x/2) keeps ACT on one table set (exp+tanh).
  - alpha1/alpha2/log_scale enter only through uploaded data, so one
    compiled program serves any input values. |alpha1*log_scale| is clamped
    to >=1e-20 when pre-dividing the mask; exact whenever alpha1*log_scale
    is not vanishingly small or the mask is zero/-inf.
"""

import sys

if "/opt/trn_rl_repo" not in sys.path:
    sys.path.insert(0, "/opt/trn_rl_repo")

from contextlib import ExitStack

import ml_dtypes
import numpy as np

import concourse.bacc as bacc
import concourse.bass as bass
import concourse.tile as tile
from concourse import mybir
from concourse.bass_utils import run_bass_kernel_spmd

B, N, M, D = 128, 512, 512, 128
NCORES = 8
BPC = B // NCORES  # batches per core
SQRT_D = float(np.sqrt(D))

F32 = mybir.dt.float32
B16 = mybir.dt.bfloat16
BF16 = ml_dtypes.bfloat16
AF = mybir.ActivationFunctionType
OP = mybir.AluOpType

_prog_cache: dict = {}

DEFAULT_CFG: dict = {}


def _build(bpc: int, repeat: int = 1, cfg: dict | None = None):
    cfg = cfg or {}
    ins_bufs = cfg.get("ins_bufs", 4)
    work_bufs = cfg.get("work_bufs", 3)
    outp_bufs = cfg.get("outp_bufs", 2)
    kv_bufs = cfg.get("kv_bufs", 2)
    nd_bufs = cfg.get("nd_bufs", 1)
    sc_bufs = cfg.get("sc_bufs", 2)
    sc_banks = cfg.get("sc_banks", 2)  # n-tiles per score PSUM tile
    fold = cfg.get("fold", "pe")  # 'pe' | 'dve': -sqrt(D)*c2*cd into score
    # DVE sums beat ACT accum_out: the accumulator needs a separate
    # ACTIVATION_READ_ACCUMULATOR (~300ns each) on the already-binding ACT.
    sums_dve = cfg.get("sums_dve", True)
    # 'dve' | 'pool': gpsimd tensor_scalar measured ~9us per 512-wide call
    # (software Q7 loop) AND produced NaNs — keep probs on DVE.
    probs_eng = cfg.get("probs_eng", "dve")
    out_bf16 = cfg.get("out_bf16", False)  # probs DMA'd out as bf16
    # no_mask: compiled variant for the (checked at runtime) case
    # ninf_mask == 0 everywhere: mask DMA/adds drop out, output identical.
    no_mask = cfg.get("no_mask", False)

    nc = bacc.Bacc(
        "TRN2",
        target_bir_lowering=False,
        debug=False,
        num_devices=NCORES,
    )

    OUT_DT = B16 if out_bf16 else F32
    # cdT (host-transposed, n slot-permuted); carries cd + mask/(-c1) when
    # a mask is present (the add is free on the host).
    cdt_d = nc.dram_tensor("cdt", (bpc, M, N), B16, kind="ExternalInput").ap()
    # natural cd (n slot-major) for the score-bias fold.
    cd_d = nc.dram_tensor("cd", (bpc, N, M), B16, kind="ExternalInput").ap()
    mask_d = (
        None
        if no_mask
        else nc.dram_tensor("maskd", (bpc, N, M), B16, kind="ExternalInput").ap()
    )
    # encT (m natural) and elnT (n slot-permuted) ride in one tensor
    # ([:, :, :M] / [:, :, M:]) so each batch needs one aux DMA.
    aux_d = nc.dram_tensor("auxT", (bpc, D, M + N), B16, kind="ExternalInput").ap()
    load_d = nc.dram_tensor("loadrow", (bpc, 1, N), B16, kind="ExternalInput").ap()
    wkv_d = nc.dram_tensor("wkvT", (D, 2 * D), B16, kind="ExternalInput").ap()
    wq1_d = nc.dram_tensor("wq1T", (D, D), B16, kind="ExternalInput").ap()
    wq2_d = nc.dram_tensor("wq2", (1, D), B16, kind="ExternalInput").ap()
    idc2_d = nc.dram_tensor("idc2", (128, 128), B16, kind="ExternalInput").ap()
    # per-partition scalars: [:, 0] = -c1 (ACT scale for eb), [:, 1] =
    # -0.1*c1 (un-scales the pre-divided mask in the logits step),
    # [:, 2] = -sqrt(D)*c2 (DVE fold variant)
    scal_d = nc.dram_tensor("scal", (128, 4), F32, kind="ExternalInput").ap()
    probs_d = nc.dram_tensor("probs", (bpc, N, M), OUT_DT, kind="ExternalOutput").ap()

    with tile.TileContext(nc) as tc, ExitStack() as ctx:
        consts = ctx.enter_context(tc.tile_pool(name="consts", bufs=1))
        ins = ctx.enter_context(tc.tile_pool(name="ins", bufs=ins_bufs))
        work = ctx.enter_context(tc.tile_pool(name="work", bufs=work_bufs))
        outp = ctx.enter_context(tc.tile_pool(name="outp", bufs=outp_bufs))
        kvp = ctx.enter_context(
            tc.tile_pool(name="kvp", bufs=kv_bufs, space=bass.MemorySpace.PSUM)
        )
        ndp = ctx.enter_context(
            tc.tile_pool(name="ndp", bufs=nd_bufs, space=bass.MemorySpace.PSUM)
        )
        scp = ctx.enter_context(
            tc.tile_pool(name="scp", bufs=sc_bufs, space=bass.MemorySpace.PSUM)
        )

        wkv_sb = consts.tile([D, 2 * D], B16)
        nc.sync.dma_start(wkv_sb, wkv_d)
        wq1_sb = consts.tile([D, D], B16)
        nc.sync.dma_start(wq1_sb, wq1_d)
        wq2_sb = consts.tile([1, D], B16)
        nc.sync.dma_start(wq2_sb, wq2_d)
        idc2_sb = consts.tile([128, 128], B16)
        nc.sync.dma_start(idc2_sb, idc2_d)
        scal_sb = consts.tile([128, 4], F32)
        nc.sync.dma_start(scal_sb, scal_d)

        def stageA(b):
            """DMA-in + everything needed before num/den: ebT, ek/ekv, sig."""
            t = {}
            # cdT: chunk c, partition p <-> m = 128c + p (natural m order);
            # free dim n is in slot order (host pre-permuted).
            cdt_t = ins.tile([128, 4, N], B16, tag="cdt")
            nc.sync.dma_start(
                cdt_t, cdt_d[b].rearrange("(c p) n -> p c n", p=128)
            )
            # natural cd / mask: slot (p, c) holds row n = 4p + c, so each
            # partition's DRAM footprint is 4 rows = 4KB contiguous.
            cd_t = ins.tile([128, 4, M], B16, tag="cd")
            nc.sync.dma_start(
                cd_t, cd_d[b].rearrange("(p c) m -> p c m", p=128)
            )
            t["cd"] = cd_t
            if not no_mask:
                mask_t = ins.tile([128, 4, M], B16, tag="mask")
                nc.sync.dma_start(
                    mask_t, mask_d[b].rearrange("(p c) m -> p c m", p=128)
                )
                t["mask"] = mask_t
            aux_t = ins.tile([D, M + N], B16, tag="auxT")
            nc.sync.dma_start(aux_t, aux_d[b])
            load_t = ins.tile([1, N], B16, tag="load")
            nc.sync.dma_start(load_t, load_d[b])
            t["encT"] = aux_t[:, :M]
            elnT_t = aux_t[:, M:]

            # ebT[m, n] = exp(-c1*(cdT + maskT/(-c1))): one 2048-wide ACT
            # exp with per-partition scale -c1, straight from the DMA tile.
            ebT_t = work.tile([128, 4, N], B16, tag="ebT")
            nc.scalar.activation(ebT_t, cdt_t, AF.Exp, scale=scal_sb[:, 0:1])
            t["ebT"] = ebT_t

            # k|v per m-chunk; ek = exp(k), ekv = ek*v (m on partitions).
            # Two 1-bank PSUM pair-tiles (not one 2-bank tile) so the q
            # matmul below can ride the same pool slot: 8 PSUM banks total
            # leaves room to double-buffer the score PSUM (sc_bufs=2).
            ek_t = work.tile([128, 4, D], B16, tag="ek")
            ekv_t = work.tile([128, 4, D], B16, tag="ekv")
            for p0 in (0, 2):
                kv_ps = kvp.tile([128, 2, 2 * D], F32, tag="kv")
                for j in range(2):
                    nc.tensor.matmul(
                        kv_ps[:, j, :],
                        t["encT"][:, (p0 + j) * 128 : (p0 + j + 1) * 128],
                        wkv_sb,
                        start=True,
                        stop=True,
                    )
                nc.scalar.activation(
                    ek_t[:, p0 : p0 + 2, :], kv_ps[:, :, 0:D], AF.Exp
                )
                nc.vector.tensor_mul(
                    ekv_t[:, p0 : p0 + 2, :],
                    ek_t[:, p0 : p0 + 2, :],
                    kv_ps[:, :, D : 2 * D],
                )
            t["ek"] = ek_t
            t["ekv"] = ekv_t

            # qT[e, n] then sigmoid via tanh: sig = 0.5*tanh(q/2) + 0.5.
            # same 2KB/partition footprint as a kv pair tile, same pool slot
            q_ps = kvp.tile([128, N], F32, tag="kv")
            nc.tensor.matmul(q_ps, wq1_sb, elnT_t, start=True, stop=False)
            nc.tensor.matmul(q_ps, wq2_sb, load_t, start=False, stop=True)
            sig_t = work.tile([128, N], F32, tag="sig")
            nc.scalar.activation(sig_t, q_ps, AF.Tanh, scale=0.5)
            nc.vector.tensor_scalar(sig_t, sig_t, 0.5, 0.5, OP.mult, OP.add)
            t["sig"] = sig_t
            return t

        def stageB(t):
            """num/den matmuls + aafm = sig * num/den."""
            nd_ps = ndp.tile([128, 2, N], F32, tag="nd")
            for mc in range(4):
                nc.tensor.matmul(
                    nd_ps[:, 0, :],
                    t["ekv"][:, mc, :],
                    t["ebT"][:, mc, :],
                    start=(mc == 0),
                    stop=(mc == 3),
                )
            for mc in range(4):
                nc.tensor.matmul(
                    nd_ps[:, 1, :],
                    t["ek"][:, mc, :],
                    t["ebT"][:, mc, :],
                    start=(mc == 0),
                    stop=(mc == 3),
                )
            # aafmT = sig * num/max(den, tiny)  (tiny clamp mirrors
            # nan_to_num for fully-masked rows: num=0 -> 0; den > 0 always
            # when there is no mask, so the clamp drops out there).
            rden_t = work.tile([128, N], F32, tag="rden")
            if no_mask:
                nc.vector.reciprocal_approx_fast(rden_t, nd_ps[:, 1, :])
            else:
                den_t = work.tile([128, N], F32, tag="den")
                nc.vector.tensor_scalar_max(den_t, nd_ps[:, 1, :], 1e-35)
                nc.vector.reciprocal_approx_fast(rden_t, den_t)
            wr_t = work.tile([128, N], F32, tag="wr")
            nc.vector.tensor_mul(wr_t, nd_ps[:, 0, :], rden_t)
            aafm_t = work.tile([128, N], B16, tag="aafm")
            nc.vector.tensor_mul(aafm_t, t["sig"], wr_t)
            t["aafm"] = aafm_t

        def stageC(b, t):
            """score + bias fold, tanh/exp softmax, probs, DMA-out."""
            probs_t = outp.tile([128, 4, M], OUT_DT, tag="probs")
            exp_t = outp.tile([128, 4, M], F32, tag="exp")
            sums_t = outp.tile([128, 4], F32, tag="sums")
            for g0 in range(0, 4, sc_banks):
                sc_ps = scp.tile([128, sc_banks, M], F32, tag="sc")
                for j in range(sc_banks):
                    nt = g0 + j
                    nc.tensor.matmul(
                        sc_ps[:, j, :],
                        t["aafm"][:, nt * 128 : (nt + 1) * 128],
                        t["encT"],
                        start=True,
                        stop=(fold != "pe"),
                    )
                if fold == "pe":
                    # consecutive idc2 matmuls share stationary weights
                    for j in range(sc_banks):
                        nt = g0 + j
                        nc.tensor.matmul(
                            sc_ps[:, j, :],
                            idc2_sb,
                            t["cd"][:, nt, :],
                            start=False,
                            stop=True,
                        )
                    tanh_in = sc_ps[:]
                else:
                    t0_t = work.tile([128, sc_banks, M], F32, tag="t0")
                    nc.vector.scalar_tensor_tensor(
                        t0_t,
                        t["cd"][:, g0 : g0 + sc_banks, :],
                        scal_sb[:, 2:3],
                        sc_ps[:],
                        OP.mult,
                        OP.add,
                    )
                    tanh_in = t0_t
                h_t = work.tile([128, sc_banks, M], F32, tag="h")
                nc.scalar.activation(h_t, tanh_in, AF.Tanh, scale=1.0 / SQRT_D)
                if no_mask:
                    u_t = h_t
                else:
                    # u = h + 0.1*mask = h + (-0.1*c1)*mask'
                    u_t = work.tile([128, sc_banks, M], F32, tag="u")
                    nc.vector.scalar_tensor_tensor(
                        u_t,
                        t["mask"][:, g0 : g0 + sc_banks, :],
                        scal_sb[:, 1:2],
                        h_t,
                        OP.mult,
                        OP.add,
                    )
                if sums_dve:
                    nc.scalar.activation(
                        exp_t[:, g0 : g0 + sc_banks, :], u_t, AF.Exp, scale=10.0
                    )
                    nc.vector.tensor_reduce(
                        sums_t[:, g0 : g0 + sc_banks],
                        exp_t[:, g0 : g0 + sc_banks, :],
                        axis=mybir.AxisListType.X,
                        op=OP.add,
                    )
                else:
                    # row sums ride the exp pass on ACT (per n-tile so the
                    # accumulator matches the softmax row)
                    for j in range(sc_banks):
                        nt = g0 + j
                        nc.scalar.activation(
                            exp_t[:, nt, :],
                            u_t[:, j, :],
                            AF.Exp,
                            scale=10.0,
                            accum_out=sums_t[:, nt : nt + 1],
                        )
            rsum_t = outp.tile([128, 4], F32, tag="rsum")
            nc.vector.reciprocal(rsum_t, sums_t)
            probs_e = nc.gpsimd if probs_eng == "pool" else nc.vector
            for nt in range(4):
                probs_e.tensor_scalar_mul(
                    probs_t[:, nt, :], exp_t[:, nt, :], rsum_t[:, nt : nt + 1]
                )
            nc.sync.dma_start(
                probs_d[b].rearrange("(p c) m -> p c m", p=128), probs_t
            )

        # Software pipeline, emission order C(i-2), A(i), B(i-1): the PE
        # queue becomes [score(i-2), kv/q(i), nd(i-1)] so every matmul's
        # inputs were produced a full stage earlier — PE never waits on the
        # DVE den-chain or ACT exp of the same batch.
        seq = [b for _ in range(repeat) for b in range(bpc)]
        state: dict = {}
        for i in range(len(seq) + 2):
            if i >= 2:
                stageC(seq[i - 2], state.pop(i - 2))
            if i < len(seq):
                state[i] = stageA(seq[i])
            if i >= 1 and i - 1 < len(seq):
                stageB(state[i - 1])

    nc.compile()
    return nc


def _get_prog(bpc: int, repeat: int = 1, cfg: dict | None = None):
    cfg = {**DEFAULT_CFG, **(cfg or {})}
    key = (bpc, repeat, tuple(sorted(cfg.items())))
    if key not in _prog_cache:
        _prog_cache[key] = _build(bpc, repeat, cfg)
    return _prog_cache[key]


def _make_in_maps(
    encoded_last_node,
    load,
    cur_dist,
    log_scale,
    ninf_mask,
    encoded_nodes,
    Wq_last,
    Wk,
    Wv,
    alpha1,
    alpha2,
    n_cores=NCORES,
):
    f = np.float32
    c1 = float(np.asarray(alpha1).reshape(-1)[0]) * float(np.asarray(log_scale))
    c2 = float(np.asarray(alpha2).reshape(-1)[0]) * float(np.asarray(log_scale))
    # mask is uploaded pre-divided by -c1 (see module docstring); clamp c1
    # away from 0 to keep that finite. Exact when mask == 0 or |c1| >= 1e-20.
    c1s = c1 if abs(c1) >= 1e-20 else (1e-20 if c1 >= 0 else -1e-20)

    # n-slot permutation (slot j holds row 4*(j%128) + j//128) to match the
    # partition-major on-chip layout of cd/mask/probs.
    perm = 4 * (np.arange(N) % 128) + np.arange(N) // 128

    cd32 = np.asarray(cur_dist, f)
    cd = cd32.astype(BF16)
    mask_np = np.asarray(ninf_mask, f)
    no_mask = not np.any(mask_np)
    if no_mask:
        maskp = None
        bias32 = cd32
    else:
        maskp = np.ascontiguousarray((mask_np / np.float32(-c1s)).astype(BF16))
        bias32 = cd32 + mask_np / np.float32(-c1s)
    # cdT[b, m, j] = bias[b, perm[j], m]  (m natural, n slot-permuted)
    cdt = np.ascontiguousarray(bias32.astype(BF16).transpose(0, 2, 1)[:, :, perm])
    cd = np.ascontiguousarray(cd)

    encT = np.asarray(encoded_nodes, f).transpose(0, 2, 1)
    elnT = np.asarray(encoded_last_node, f).transpose(0, 2, 1)[:, :, perm]
    auxT = np.ascontiguousarray(
        np.concatenate([encT, elnT], axis=2).astype(BF16)
    )
    loadrow = np.ascontiguousarray(
        np.asarray(load, f)[:, perm].reshape(B, 1, N).astype(BF16)
    )

    Wq = np.asarray(Wq_last, f)
    wkvT = np.ascontiguousarray(
        np.concatenate([np.asarray(Wk, f).T, np.asarray(Wv, f).T], axis=1).astype(BF16)
    )
    wq1T = np.ascontiguousarray(Wq[:, :D].T.astype(BF16))
    wq2 = np.ascontiguousarray(Wq[:, D : D + 1].T.astype(BF16))

    scal = np.zeros((128, 4), f)
    scal[:, 0] = -c1s
    scal[:, 1] = -0.1 * c1s
    scal[:, 2] = -SQRT_D * c2
    shared = {
        "wkvT": wkvT,
        "wq1T": wq1T,
        "wq2": wq2,
        "idc2": np.ascontiguousarray(
            ((-SQRT_D * c2) * np.eye(128)).astype(BF16)
        ),
        "scal": scal,
    }

    bpc = B // n_cores
    in_maps = []
    for i in range(n_cores):
        sl = slice(i * bpc, (i + 1) * bpc)
        m = {
            "cdt": cdt[sl],
            "cd": cd[sl],
            "auxT": auxT[sl],
            "loadrow": loadrow[sl],
            **shared,
        }
        if not no_mask:
            m["maskd"] = maskp[sl]
        in_maps.append(m)
    return in_maps, no_mask


def _run(trace=False, repeat=1, cfg=None, **inputs):
    """Build + run on 8 cores; returns (probs, BassKernelResults)."""
    in_maps, no_mask = _make_in_maps(**inputs)
    cfg = {**DEFAULT_CFG, **(cfg or {})}
    cfg["no_mask"] = no_mask
    nc = _get_prog(BPC, repeat, cfg)
    res = run_bass_kernel_spmd(nc, in_maps, core_ids=list(range(NCORES)), trace=trace)
    probs = np.concatenate([r["probs"] for r in res.results], axis=0)
    return np.ascontiguousarray(probs.astype(np.float32)), res


def kernel(**inputs) -> np.ndarray:
    probs, _ = _run(trace=False, **inputs)
    return probs


if __name__ == "__main__":
    rng = np.random.default_rng(0)
    demo = {
        "encoded_last_node": rng.standard_normal((B, N, D), dtype=np.float32),
        "load": rng.random((B, N), dtype=np.float32),
        "cur_dist": rng.random((B, N, M), dtype=np.float32),
        "log_scale": np.ones((), np.float32),
        "ninf_mask": np.zeros((B, N, M), np.float32),
        "encoded_nodes": rng.standard_normal((B, M, D), dtype=np.float32),
        "Wq_last": rng.standard_normal((D, D + 1), dtype=np.float32) / SQRT_D,
        "Wk": rng.standard_normal((D, D), dtype=np.float32) / SQRT_D,
        "Wv": rng.standard_normal((D, D), dtype=np.float32) / SQRT_D,
        "alpha1": np.ones((1,), np.float32),
        "alpha2": np.ones((1,), np.float32),
    }
    out = kernel(**demo)
    print("kernel output", out.shape, out.dtype, out.sum())
